# revision 4
# baseline (speedup 1.0000x reference)
"""Self-contained Trainium2 Bass kernel: UR5 DH forward kinematics (position).

kernel(joint_angles [1048576,6] f32, dh_params [6,4] f32) -> [1048576,3] f32

Sharding: pure data parallel — batch split evenly across 8 NeuronCores;
dh_params is folded into compile-time scalar constants (the DH table's theta
offsets are all zero and a6=0, so the position reduces to a closed form).

Closed form (algebraically identical to chaining the six 4x4 DH transforms
and reading T[:3,3]; verified to fp64 round-off against the matrix chain):
  q23 = q2+q3 ; q234 = q23+q4
  Y  = -d6*s5*s234 - d5*c234 + a3*s23 + a2*s2        (pz = Y + d1)
  X  = -d6*s5*c234 + d5*s234 + a3*c23 + a2*c2
  v2 = d6*c5 + d4
  px = c1*X + s1*v2 ; py = s1*X - c1*v2

The HW ACT Sin spline is only accurate on ~[-pi,pi]; inputs reach ~8.6 rad,
so every angle is range-reduced with the fp32 magic-number rounding trick:
  t2 = q*(1/2pi) + 1.5*2^23 ; k2p = (t2 - 1.5*2^23)*2pi ; r' = k2p - q
giving r' = -wrap(q) in [-pi,pi]; then sin(q) = Sin(-r'),
cos(q) = Sin(pi/2 - |r'|), -cos(q) = Sin(|r'| - pi/2)  (|.| via sign-bit AND).
"""
import math
from contextlib import nullcontext

import numpy as np

import concourse.bass as bass
import concourse.mybir as mybir
from concourse.tile import TileContext
from concourse import tile as _tile
from concourse import bass_utils

F32 = mybir.dt.float32
PI = math.pi
TWO_PI = 2.0 * math.pi
INV_2PI = 1.0 / TWO_PI
HALF_PI = 0.5 * math.pi
MAGIC = 1.5 * 2.0**23

P = 128
N_CORES = 8
B_TOTAL = 1048576
B_CORE = B_TOTAL // N_CORES
N_CHUNKS = 4

# ---------------------------------------------------------------------------
# This container's walrus build encodes at most ONE semaphore wait per
# instruction. Two fixups: (a) the TileContext exit drain gets one wait per
# DMA-sem lane -> split across several drains; (b) Tile's scheduler can attach
# two waits to a compute instruction -> hoist extras onto standalone
# same-engine EventSemaphore carriers placed just before it.
# ---------------------------------------------------------------------------


def _patched_drain_and_barrier(self, tick_clock, wait_clock):
    nc = self.nc
    carrier = nc.sync.drain()
    wait_clock.add_sem_waits(
        carrier.ins, _tile.ScopedClock({None: tick_clock.global_clock})
    )
    si = carrier.ins.sync_info
    if si is not None and len(si.on_wait) > 1:
        waits = list(si.on_wait)
        carrier.ins.sync_info = mybir.SyncInfo(on_wait=[waits[0]], on_update=[])
        for w in waits[1:]:
            extra = nc.sync.drain()
            extra.ins.sync_info = mybir.SyncInfo(on_wait=[w], on_update=[])

    nc.all_engine_barrier()
    assert self.sems is not None
    popped = nc._tile_sem_poison_stack.pop()
    assert popped is self._sem_poison
    nc.clear_and_free_semaphores(list(self.sems.allocated().values()))
    nc.all_engine_barrier()


_tile.TileContext._drain_and_barrier = _patched_drain_and_barrier

_split_counter = [0]


def _split_multi_waits(nc):
    for func in nc.m.functions:
        for bb in func.blocks:
            insts = bb.instructions
            new_list = []
            changed = False
            for inst in insts:
                si = inst.sync_info
                waits = list(si.on_wait) if si is not None else []
                if len(waits) > 1:
                    changed = True
                    for w in waits[:-1]:
                        _split_counter[0] += 1
                        carrier = mybir.InstEventSemaphore(
                            name=f"WSPLIT-{_split_counter[0]}", ins=[], outs=[])
                        carrier.engine = inst.engine
                        carrier.sync_info = mybir.SyncInfo(on_wait=[w], on_update=[])
                        new_list.append(carrier)
                    inst.sync_info = mybir.SyncInfo(
                        on_wait=[waits[-1]], on_update=list(si.on_update))
                new_list.append(inst)
            if changed:
                bb.instructions = new_list


def _build_fk_nc(b_core: int, dh: np.ndarray, n_chunks: int = N_CHUNKS,
                 repeat: int = 1):
    d1 = float(dh[0, 1]); a2 = float(dh[1, 2]); a3 = float(dh[2, 2])
    d4 = float(dh[3, 1]); d5 = float(dh[4, 1]); d6 = float(dh[5, 1])

    assert b_core % P == 0
    ncol = b_core // P
    assert ncol % n_chunks == 0
    n = ncol // n_chunks

    nc = bass.Bass("TRN2")
    ja = nc.dram_tensor("ja", [b_core, 6], F32, kind="ExternalInput")
    out = nc.dram_tensor("pos", [b_core, 3], F32, kind="ExternalOutput")

    halfpi_t = nc.alloc_sbuf_tensor("halfpi", [P, 1], F32)
    neghalfpi_t = nc.alloc_sbuf_tensor("neghalfpi", [P, 1], F32)
    nc.gpsimd.memset(halfpi_t.ap(), HALF_PI)
    nc.gpsimd.memset(neghalfpi_t.ap(), -HALF_PI)
    nc.all_engine_barrier()
    halfpi = halfpi_t.ap()
    neghalfpi = neghalfpi_t.ap()

    ja3 = ja[:].rearrange("(p n) c -> p n c", p=P)
    out3 = out[:].rearrange("(p n) c -> p n c", p=P)

    Sin = mybir.ActivationFunctionType.Sin
    ADD = mybir.AluOpType.add
    SUB = mybir.AluOpType.subtract
    MULT = mybir.AluOpType.mult
    BAND = mybir.AluOpType.bitwise_and
    U32 = mybir.dt.uint32

    def emit_chunk(pool, ci):
        sl = slice(ci * n, (ci + 1) * n)
        t_in = pool.tile([P, n, 6], F32, tag="in")
        nc.sync.dma_start(out=t_in[:], in_=ja3[:, sl, :])

        q2t = pool.tile([P, 2, n], F32, tag="q2t")   # [q23 | q234]
        t2a = pool.tile([P, 2, n], F32, tag="t2a")
        t2b = pool.tile([P, 2, n], F32, tag="t2b")
        t2c = pool.tile([P, 1, n], F32, tag="t2c")
        ra = pool.tile([P, 2, n], F32, tag="ra")     # [-r1 | -r2]
        rb = pool.tile([P, 2, n], F32, tag="rb")     # [-r23 | -r234]
        rc = pool.tile([P, 1, n], F32, tag="rc")     # [-r5]
        ua = pool.tile([P, 2, n], F32, tag="ua")
        ub = pool.tile([P, 2, n], F32, tag="ub")
        uc = pool.tile([P, 1, n], F32, tag="uc")
        t1 = pool.tile([P, 5, n], F32, tag="t1")     # [s23|c23|c234|c234n|s234]
        t2 = pool.tile([P, 5, n], F32, tag="t2")     # [c1|s1|c1xn|s2|c2]
        sc5 = pool.tile([P, 2, n], F32, tag="sc5")   # [s5|c5]
        xy = pool.tile([P, 2, n], F32, tag="xy")     # [Y|X]
        xy2 = pool.tile([P, 2, n], F32, tag="xy2")
        xy3 = pool.tile([P, 2, n], F32, tag="xy3")
        tv = pool.tile([P, 3, n], F32, tag="tv")     # [Ypre|v0|v2]
        p4 = pool.tile([P, 4, n], F32, tag="p4")     # [u1|u2|u3|u4]
        t_out = pool.tile([P, n, 3], F32, tag="out")

        in_q12 = t_in[:, :, 0:2].transpose([0, 2, 1])   # [P,2,n] strided
        in_q5 = t_in[:, :, 4]                           # [P,n] strided

        nc.vector.tensor_tensor(q2t[:, 0], t_in[:, :, 1], t_in[:, :, 2], ADD)
        nc.vector.tensor_tensor(q2t[:, 1], q2t[:, 0], t_in[:, :, 3], ADD)

        # range reduction
        nc.gpsimd.tensor_scalar(t2a[:], in_q12, INV_2PI, MAGIC, MULT, ADD)
        nc.gpsimd.tensor_scalar(t2b[:], q2t[:], INV_2PI, MAGIC, MULT, ADD)
        nc.gpsimd.tensor_scalar(t2c[:, 0], in_q5, INV_2PI, MAGIC, MULT, ADD)
        nc.gpsimd.tensor_scalar(t2a[:], t2a[:], MAGIC, TWO_PI, SUB, MULT)
        nc.gpsimd.tensor_scalar(t2b[:], t2b[:], MAGIC, TWO_PI, SUB, MULT)
        nc.gpsimd.tensor_scalar(t2c[:], t2c[:], MAGIC, TWO_PI, SUB, MULT)
        nc.vector.tensor_tensor(ra[:], t2a[:], in_q12, SUB)
        nc.vector.tensor_tensor(rb[:], t2b[:], q2t[:], SUB)
        nc.vector.tensor_tensor(rc[:, 0], t2c[:, 0], in_q5, SUB)
        nc.vector.tensor_scalar(ua[:].bitcast(U32), ra[:].bitcast(U32),
                                0x7FFFFFFF, None, BAND)
        nc.vector.tensor_scalar(ub[:].bitcast(U32), rb[:].bitcast(U32),
                                0x7FFFFFFF, None, BAND)
        nc.vector.tensor_scalar(uc[:].bitcast(U32), rc[:].bitcast(U32),
                                0x7FFFFFFF, None, BAND)

        # trig (ACT runs ONLY Sin -> single table set, no reload thrash)
        def sin_of(o, i):
            nc.scalar.activation(o, i, Sin, scale=-1.0)

        def cos_of(o, u):
            nc.scalar.activation(o, u, Sin, bias=halfpi, scale=-1.0)

        # paired by identical (func, scale, bias); outputs step-sliced
        sin_of(t1[:, 0:5:4], rb[:])       # [s23 | s234] -> cols {0,4}
        cos_of(t1[:, 1:3], ub[:])         # [c23 | c234] -> cols {1,2}
        cos_of(t2[:, 0:5:4], ua[:])       # [c1 | c2]   -> cols {0,4}
        sin_of(t2[:, 1:4:2], ra[:])       # [s1 | s2]   -> cols {1,3}
        sin_of(sc5[:, 0], rc[:, 0])       # s5
        cos_of(sc5[:, 1], uc[:, 0])       # c5

        # chain: [Y|X]
        nc.vector.scalar_tensor_tensor(xy[:, 0], t1[:, 4], -d6, sc5[:, 0],
                                       MULT, MULT)
        nc.vector.scalar_tensor_tensor(xy[:, 1], t1[:, 2], -d6, sc5[:, 0],
                                       MULT, MULT)
        nc.vector.scalar_tensor_tensor(xy2[:, 0], t1[:, 2], -d5, xy[:, 0],
                                       MULT, ADD)
        nc.vector.scalar_tensor_tensor(xy2[:, 1], t1[:, 4], d5, xy[:, 1],
                                       MULT, ADD)
        nc.vector.scalar_tensor_tensor(xy3[:], t1[:, 0:2], a3, xy2[:],
                                       MULT, ADD)
        nc.vector.scalar_tensor_tensor(tv[:, 0:2], t2[:, 3:5], a2, xy3[:],
                                       MULT, ADD)
        nc.gpsimd.tensor_scalar(tv[:, 2], sc5[:, 1], d6, d4, MULT, ADD)

        # rotation by q1
        nc.vector.tensor_tensor(p4[:, 0:2], t2[:, 0:2], tv[:, 1:3], MULT)
        nc.vector.tensor_tensor(p4[:, 2], t2[:, 1], tv[:, 1], MULT)
        nc.vector.tensor_tensor(p4[:, 3], t2[:, 0], tv[:, 2], MULT)
        nc.gpsimd.tensor_tensor(t_out[:, :, 0], p4[:, 0], p4[:, 1], ADD)
        nc.gpsimd.tensor_tensor(t_out[:, :, 1], p4[:, 2], p4[:, 3], SUB)
        nc.gpsimd.tensor_scalar(t_out[:, :, 2], tv[:, 0], d1, None, ADD)

        nc.sync.dma_start(out=out3[:, sl, :], in_=t_out[:])

    with TileContext(nc) as tc:
        with tc.tile_pool(name="fk", bufs=2) as pool:
            with (tc.For_i(0, repeat) if repeat > 1 else nullcontext()):
                for ci in range(n_chunks):
                    emit_chunk(pool, ci)

    _split_multi_waits(nc)
    return nc


_NC_CACHE: dict[tuple, object] = {}


def kernel(joint_angles: np.ndarray, dh_params: np.ndarray) -> np.ndarray:
    ja = np.ascontiguousarray(np.asarray(joint_angles, dtype=np.float32))
    dh = np.asarray(dh_params, dtype=np.float64)
    B = ja.shape[0]
    assert B % N_CORES == 0
    b_core = B // N_CORES

    key = (b_core, dh.tobytes())
    nc = _NC_CACHE.get(key)
    if nc is None:
        nc = _build_fk_nc(b_core, dh)
        _NC_CACHE[key] = nc

    in_maps = [{"ja": np.ascontiguousarray(ja[i * b_core:(i + 1) * b_core])}
               for i in range(N_CORES)]
    res = bass_utils.run_bass_kernel_spmd(nc, in_maps, core_ids=list(range(N_CORES)))
    return np.concatenate([r["pos"] for r in res.results], axis=0)


# revision 5
# speedup vs baseline: 1.3508x; 1.3508x over previous
"""Self-contained Trainium2 Bass kernel: UR5 DH forward kinematics (position).

kernel(joint_angles [1048576,6] f32, dh_params [6,4] f32) -> [1048576,3] f32

Sharding: pure data parallel — batch split evenly across 8 NeuronCores;
dh_params is folded into compile-time scalar constants (the DH table's theta
offsets are all zero and a6=0, so the position reduces to a closed form).

Closed form (algebraically identical to chaining the six 4x4 DH transforms
and reading T[:3,3]; verified to fp64 round-off against the matrix chain):
  q23 = q2+q3 ; q234 = q23+q4
  Y  = -d6*s5*s234 - d5*c234 + a3*s23 + a2*s2        (pz = Y + d1)
  X  = -d6*s5*c234 + d5*s234 + a3*c23 + a2*c2
  v2 = d6*c5 + d4
  px = c1*X + s1*v2 ; py = s1*X - c1*v2

The HW ACT Sin spline is only accurate on [-pi,pi] (measured); inputs reach
~8.6 rad, so each angle is range-reduced with the fp32 magic-number rounding
trick, split across engines to avoid the (slow) gpsimd engine entirely:
  ACT:  m = q*(1/2pi) + 1.5*2^23          (Copy activation, single-rounded)
  DVE:  w = (m - 1.5*2^23) * 2pi          (exact k*2pi by Sterbenz)
  DVE:  r = q - w                         (= wrap(q) in [-pi,pi])
  ACT:  S = Sin(r); U = Abs(r); C = Sin(pi/2 - U); extra cols -cos via
        Sin(U - pi/2).
The post-trig chain runs on DVE in fp16 (tolerance is 2e-2 rel; fp16 adds
~1e-3 abs) using column-pair access patterns: the [Y|X] accumulation and the
final q1 rotation are 2-wide ops over strided column pairs of one SC tile,
with per-row multipliers broadcast via stride-0 APs.
"""
import math
from contextlib import nullcontext

import numpy as np

import concourse.bass as bass
import concourse.mybir as mybir
from concourse.tile import TileContext
from concourse import tile as _tile
from concourse import bass_utils

F32 = mybir.dt.float32
F16 = mybir.dt.float16
PI = math.pi
TWO_PI = 2.0 * math.pi
INV_2PI = 1.0 / TWO_PI
HALF_PI = 0.5 * math.pi
MAGIC = 1.5 * 2.0**23

P = 128
N_CORES = 8
B_TOTAL = 1048576
B_CORE = B_TOTAL // N_CORES
N_CHUNKS = 2

# ---------------------------------------------------------------------------
# This container's walrus build encodes at most ONE semaphore wait per
# instruction. Two fixups: (a) the TileContext exit drain gets one wait per
# DMA-sem lane -> split across several drains; (b) Tile's scheduler can attach
# two waits to a compute instruction -> hoist extras onto standalone
# same-engine EventSemaphore carriers placed just before it.
# ---------------------------------------------------------------------------


def _patched_drain_and_barrier(self, tick_clock, wait_clock):
    nc = self.nc
    carrier = nc.sync.drain()
    wait_clock.add_sem_waits(
        carrier.ins, _tile.ScopedClock({None: tick_clock.global_clock})
    )
    si = carrier.ins.sync_info
    if si is not None and len(si.on_wait) > 1:
        waits = list(si.on_wait)
        carrier.ins.sync_info = mybir.SyncInfo(on_wait=[waits[0]], on_update=[])
        for w in waits[1:]:
            extra = nc.sync.drain()
            extra.ins.sync_info = mybir.SyncInfo(on_wait=[w], on_update=[])

    nc.all_engine_barrier()
    assert self.sems is not None
    popped = nc._tile_sem_poison_stack.pop()
    assert popped is self._sem_poison
    nc.clear_and_free_semaphores(list(self.sems.allocated().values()))
    nc.all_engine_barrier()


_tile.TileContext._drain_and_barrier = _patched_drain_and_barrier

_split_counter = [0]


def _split_multi_waits(nc):
    for func in nc.m.functions:
        for bb in func.blocks:
            insts = bb.instructions
            new_list = []
            changed = False
            for inst in insts:
                si = inst.sync_info
                waits = list(si.on_wait) if si is not None else []
                if len(waits) > 1:
                    changed = True
                    for w in waits[:-1]:
                        _split_counter[0] += 1
                        carrier = mybir.InstEventSemaphore(
                            name=f"WSPLIT-{_split_counter[0]}", ins=[], outs=[])
                        carrier.engine = inst.engine
                        carrier.sync_info = mybir.SyncInfo(on_wait=[w], on_update=[])
                        new_list.append(carrier)
                    inst.sync_info = mybir.SyncInfo(
                        on_wait=[waits[-1]], on_update=list(si.on_update))
                new_list.append(inst)
            if changed:
                bb.instructions = new_list


def _build_fk_nc(b_core: int, dh: np.ndarray, n_chunks: int = N_CHUNKS,
                 repeat: int = 1, use_f16: bool = True):
    d1 = float(dh[0, 1]); a2 = float(dh[1, 2]); a3 = float(dh[2, 2])
    d4 = float(dh[3, 1]); d5 = float(dh[4, 1]); d6 = float(dh[5, 1])

    assert b_core % P == 0
    ncol = b_core // P
    assert ncol % n_chunks == 0
    n = ncol // n_chunks

    FC = F16 if use_f16 else F32

    nc = bass.Bass("TRN2")
    ja = nc.dram_tensor("ja", [b_core, 6], F32, kind="ExternalInput")
    out = nc.dram_tensor("pos", [b_core, 3], F32, kind="ExternalOutput")

    halfpi_t = nc.alloc_sbuf_tensor("halfpi", [P, 1], F32)
    neghalfpi_t = nc.alloc_sbuf_tensor("neghalfpi", [P, 1], F32)
    nc.gpsimd.memset(halfpi_t.ap(), HALF_PI)
    nc.gpsimd.memset(neghalfpi_t.ap(), -HALF_PI)
    nc.all_engine_barrier()
    halfpi = halfpi_t.ap()
    neghalfpi = neghalfpi_t.ap()

    ja3 = ja[:].rearrange("(p n) c -> p n c", p=P)
    out3 = out[:].rearrange("(p n) c -> p n c", p=P)

    Sin = mybir.ActivationFunctionType.Sin
    Abs = mybir.ActivationFunctionType.Abs
    Copy = mybir.ActivationFunctionType.Copy
    ADD = mybir.AluOpType.add
    SUB = mybir.AluOpType.subtract
    MULT = mybir.AluOpType.mult

    # SC tile columns: sin(q1,q2,q5,q23,q234), cos(q1,q2,q5,q23,q234),
    # -cos(q234), -cos(q1)
    S1, S2, S5, S23, S234 = 0, 1, 2, 3, 4
    C1, C2, C5, C23, C234 = 5, 6, 7, 8, 9
    NC234, NC1 = 10, 11

    def emit_chunk(pool, ci):
        sl = slice(ci * n, (ci + 1) * n)
        t_in = pool.tile([P, n, 6], F32, tag="in")
        nc.sync.dma_start(out=t_in[:], in_=ja3[:, sl, :])

        in_q12 = t_in[:, :, 0:2].transpose([0, 2, 1])   # [P,2,n] stride-6
        in_q5 = t_in[:, :, 4]                           # [P,n] stride-6

        Q = pool.tile([P, 2, n], F32, tag="Q")          # [q23 | q234]
        M = pool.tile([P, 5, n], F32, tag="M")
        W = pool.tile([P, 5, n], F32, tag="W")
        R = pool.tile([P, 5, n], F32, tag="R")
        U = pool.tile([P, 5, n], F32, tag="U")
        SC = pool.tile([P, 12, n], FC, tag="SC")
        w5 = pool.tile([P, 1, n], FC, tag="w5")
        acc = pool.tile([P, 2, n], FC, tag="acc")
        acc2 = pool.tile([P, 2, n], FC, tag="acc2")
        acc3 = pool.tile([P, 2, n], FC, tag="acc3")
        p4 = pool.tile([P, 2, n], FC, tag="p4")
        xy = pool.tile([P, 2, n], FC, tag="xy")
        v2 = pool.tile([P, 1, n], FC, tag="v2")
        m1 = pool.tile([P, 2, n], FC, tag="m1")
        m2 = pool.tile([P, 2, n], FC, tag="m2")
        t_out = pool.tile([P, n, 3], F32, tag="out")

        # q23, q234 (DVE, strided input reads)
        nc.vector.tensor_tensor(Q[:, 0], t_in[:, :, 1], t_in[:, :, 2], ADD)
        nc.vector.tensor_tensor(Q[:, 1], Q[:, 0], t_in[:, :, 3], ADD)

        # m = q/2pi + MAGIC  (ACT Copy: single-rounded FMA keeps the trick)
        nc.scalar.activation(M[:, 0:2], in_q12, Copy, bias=MAGIC, scale=INV_2PI)
        nc.scalar.activation(M[:, 2], in_q5, Copy, bias=MAGIC, scale=INV_2PI)
        nc.scalar.activation(M[:, 3:5], Q[:], Copy, bias=MAGIC, scale=INV_2PI)

        # w = (m - MAGIC)*2pi ; r = q - w = wrap(q)  (DVE)
        nc.vector.tensor_scalar(W[:], M[:], MAGIC, TWO_PI, SUB, MULT)
        nc.vector.tensor_tensor(R[:, 0:2], in_q12, W[:, 0:2], SUB)
        nc.vector.tensor_tensor(R[:, 2], in_q5, W[:, 2], SUB)
        nc.vector.tensor_tensor(R[:, 3:5], Q[:], W[:, 3:5], SUB)

        # trig (ACT): S = Sin(r); U = |r|; C = Sin(pi/2 - U); negated cos cols
        nc.scalar.activation(U[:], R[:], Abs)
        nc.scalar.activation(SC[:, 0:5], R[:], Sin)
        nc.scalar.activation(SC[:, 5:10], U[:], Sin, bias=halfpi, scale=-1.0)
        nc.scalar.activation(SC[:, NC234], U[:, 4], Sin, bias=neghalfpi,
                             scale=1.0)
        nc.scalar.activation(SC[:, NC1], U[:, 0], Sin, bias=neghalfpi,
                             scale=1.0)

        # chain (DVE, fp16): [Y|X] pair ops over SC column pairs
        nc.vector.tensor_scalar(w5[:, 0], SC[:, S5], -d6, None, MULT)
        nc.vector.tensor_scalar(acc[:], SC[:, S2:C2 + 1:5], a2, None, MULT)
        nc.vector.scalar_tensor_tensor(acc2[:], SC[:, S23:C23 + 1:5], a3,
                                       acc[:], MULT, ADD)
        nc.vector.scalar_tensor_tensor(acc3[:], SC[:, NC234:S234 - 1:-6], d5,
                                       acc2[:], MULT, ADD)
        nc.vector.tensor_tensor(p4[:], SC[:, S234:C234 + 1:5],
                                w5[:].broadcast_to([P, 2, n]), MULT)
        nc.vector.tensor_tensor(xy[:], acc3[:], p4[:], ADD)      # [Y|X]
        nc.vector.tensor_scalar(v2[:, 0], SC[:, C5], d6, d4, MULT, ADD)
        nc.vector.tensor_scalar(t_out[:, :, 2], xy[:, 0], d1, None, ADD)
        nc.vector.tensor_tensor(m1[:], SC[:, C1::-5],
                                xy[:, 1:2].broadcast_to([P, 2, n]), MULT)
        nc.vector.tensor_tensor(m2[:], SC[:, S1:NC1 + 1:11],
                                v2[:].broadcast_to([P, 2, n]), MULT)
        nc.vector.tensor_tensor(t_out[:, :, 0:2].transpose([0, 2, 1]),
                                m1[:], m2[:], ADD)

        nc.sync.dma_start(out=out3[:, sl, :], in_=t_out[:])

    with TileContext(nc) as tc:
        with tc.tile_pool(name="fk", bufs=2) as pool:
            with (tc.For_i(0, repeat) if repeat > 1 else nullcontext()):
                for ci in range(n_chunks):
                    emit_chunk(pool, ci)

    _split_multi_waits(nc)
    return nc


_NC_CACHE: dict[tuple, object] = {}


def kernel(joint_angles: np.ndarray, dh_params: np.ndarray) -> np.ndarray:
    ja = np.ascontiguousarray(np.asarray(joint_angles, dtype=np.float32))
    dh = np.asarray(dh_params, dtype=np.float64)
    B = ja.shape[0]
    assert B % N_CORES == 0
    b_core = B // N_CORES

    key = (b_core, dh.tobytes())
    nc = _NC_CACHE.get(key)
    if nc is None:
        nc = _build_fk_nc(b_core, dh)
        _NC_CACHE[key] = nc

    in_maps = [{"ja": np.ascontiguousarray(ja[i * b_core:(i + 1) * b_core])}
               for i in range(N_CORES)]
    res = bass_utils.run_bass_kernel_spmd(nc, in_maps, core_ids=list(range(N_CORES)))
    return np.concatenate([r["pos"] for r in res.results], axis=0)
